# revision 37
# baseline (speedup 1.0000x reference)
"""Trainium2 Bass kernel for ContourIntegrationLayer.

Reference computation (per batch element, fp32):
    conv = depthwise_conv2d(x, kernel, 5x5, SAME zero-pad)   # per-channel
    y    = (conv * alpha + bias) * x + x

Sharding: pure data parallel over the batch dim (32 -> 4 images per core
across 8 cores); kernel/alpha/bias are tiny and folded host-side:
    kv[c, t] = alpha * kernel[t_r, t_c, c]      (per-channel tap weights)
    cb       = bias + 1                          (gate offset)
so on-device:  y = (depthwise_conv(x, kv) + cb) * x.

Device layout: the core's 4 images x 96 channels = 384 channel-planes are
packed onto 3 groups of 128 SBUF partitions (plane F = g*128 + p maps to
image F//96, channel F%96).  Per group, the zero-padded spatial plane is
flattened along the free dim and each of the 25 taps is one op over a
whole strip, using the padded-plane trick (junk pad columns computed but
never read):
  - DVE: scalar_tensor_tensor  acc = x_shift * kv[t] + acc  (front region)
  - PE:  diag(kv[:, t]) @ x_shift accumulated in PSUM, float32r (tail)
NHWC <-> channel-major transposes run on the PE via identity matmul.
Emission is software-pipelined (load s+1 | conv s | gate/store s-1) so no
engine's in-order queue stalls on a cross-stage dependency.
"""

import numpy as np
from contextlib import ExitStack

import concourse.bass as bass
import concourse.tile as tile
from concourse import bacc, masks, mybir
from concourse.bass_utils import run_bass_kernel_spmd

F32 = mybir.dt.float32

B, H, W, CH, N = 32, 112, 112, 96, 5
NCORES = 8
IMG = B // NCORES          # images per core (4)
NGRP = IMG * CH // 128     # partition groups (3)
PAD = N // 2               # 2
WP = W + 2 * PAD           # 116 padded row width
SH = 56                    # output rows per strip
NSTRIP = H // SH           # 2
SHI = SH + 2 * PAD         # input rows per strip (60)
LS = (SH - 1) * WP + W     # flat madd length per strip (6492)
QR = 15                    # staged rows per load chunk

# group g covers channel-planes [g*128, (g+1)*128): segments of
# (img, ch_lo, ch_hi, col_lo) with col = partition slot within the group
SEGS = []
for g in range(NGRP):
    segs, f = [], g * 128
    while f < (g + 1) * 128:
        img, ch = f // CH, f % CH
        n = min(CH - ch, (g + 1) * 128 - f)
        segs.append((img, ch, ch + n, f - g * 128))
        f += n
    SEGS.append(segs)

# flat elems per strip offloaded to PE diag-matmuls (multiple of 512)
PE_LEN = 4608
PE_F32R = True


def _build_program(pe_len=None, pe_f32r=None):
    pe_len = PE_LEN if pe_len is None else pe_len
    pe_f32r = PE_F32R if pe_f32r is None else pe_f32r
    nc = bacc.Bacc("TRN2", target_bir_lowering=False, debug=False,
                   num_devices=NCORES)
    x_d = nc.dram_tensor("x", [IMG, H, W, CH], F32, kind="ExternalInput").ap()
    kv_d = nc.dram_tensor("kv", [NGRP, 128, N * N], F32,
                          kind="ExternalInput").ap()
    cb_d = nc.dram_tensor("cb", [128, 1], F32, kind="ExternalInput").ap()
    y_d = nc.dram_tensor("y", [IMG, H, W, CH], F32, kind="ExternalOutput").ap()

    with tile.TileContext(nc) as tc:
        _kernel(tc, y_d, x_d, kv_d, cb_d, pe_len, pe_f32r)
    nc.compile()
    return nc


def _kernel(tc, y_d, x_d, kv_d, cb_d, pe_len, pe_f32r):
    nc = tc.nc
    ctx = ExitStack()
    const_pool = ctx.enter_context(tc.tile_pool(name="const", bufs=1))
    diag_pool = ctx.enter_context(tc.tile_pool(name="diag", bufs=2))
    xpad_pool = ctx.enter_context(tc.tile_pool(name="xpad", bufs=3))
    acc_pool = ctx.enter_context(tc.tile_pool(name="acc", bufs=2))
    staged_pool = ctx.enter_context(tc.tile_pool(name="staged", bufs=2))
    ostage_pool = ctx.enter_context(tc.tile_pool(name="ostage", bufs=3))
    psin_pool = ctx.enter_context(tc.tile_pool(name="psin", bufs=2,
                                               space="PSUM"))
    psout_pool = ctx.enter_context(tc.tile_pool(name="psout", bufs=2,
                                                space="PSUM"))

    ident = const_pool.tile([128, 128], F32)
    masks.make_identity(nc, ident[:])
    kvg = const_pool.tile([128, NGRP, N * N], F32)
    for g in range(NGRP):
        nc.sync.dma_start(out=kvg[:, g, :], in_=kv_d[g])
    cb = const_pool.tile([128, 1], F32)
    nc.sync.dma_start(out=cb[:], in_=cb_d[:, :])

    mmdt = mybir.dt.float32r if pe_f32r else F32
    if pe_len:
        pacc_pool = ctx.enter_context(
            tc.tile_pool(name="pacc", bufs=3, space="PSUM"))
        xr_pool = ctx.enter_context(tc.tile_pool(name="xr", bufs=2))

    diag_cache = {}

    def get_diag(g):
        # per-tap diagonal weight matrices for group g; two slots rotate so
        # the next group's set is built while the current one is in use
        if g not in diag_cache:
            diag = diag_pool.tile([128, N * N, 128], mmdt, name="diag",
                                  tag="diag")
            for t in range(N * N):
                nc.vector.tensor_scalar_mul(
                    diag[:, t, :], ident[:], kvg[:, g, t:t + 1])
            diag_cache[g] = diag
            if len(diag_cache) > 2:
                diag_cache.pop(min(diag_cache))
        return diag_cache[g]

    def load_stage(g, s):
        """DMA + PE-transpose one strip of both images into plane-major."""
        r0 = SH * s - PAD  # first padded input row (may be <0)
        xpad = xpad_pool.tile([128, SHI, WP], F32, name="xpad", tag="xpad")

        # zero the left/right padding columns, and any out-of-image rows
        nc.gpsimd.memset(xpad[:, :, 0:PAD], 0.0)
        nc.gpsimd.memset(xpad[:, :, W + PAD:WP], 0.0)
        if r0 < 0:
            nc.gpsimd.memset(xpad[:, 0:-r0, :], 0.0)
        if r0 + SHI > H:
            nc.gpsimd.memset(xpad[:, H - r0:SHI, :], 0.0)

        for q0 in range(0, SHI, QR):
            h_lo = r0 + q0                       # first padded row this chunk
            rows = [r for r in range(h_lo, h_lo + QR) if 0 <= r < H]
            if not rows:
                continue
            ra, rb = rows[0], rows[-1] + 1
            staged = staged_pool.tile([W, QR, 128], F32, name="staged",
                                      tag="staged")
            for img, ch_lo, ch_hi, col in SEGS[g]:
                nc.sync.dma_start(
                    out=staged[:, 0:rb - ra, col:col + ch_hi - ch_lo],
                    in_=x_d[img, ra:rb, :, ch_lo:ch_hi].rearrange(
                        "h w c -> w h c"),
                )
            # transpose rows in groups of 4 -> one PSUM bank each
            for gr in range(0, rb - ra, 4):
                gn = min(4, rb - ra - gr)
                psin = psin_pool.tile([128, 4, W], F32, name="psin",
                                      tag="psin")
                for j in range(gn):
                    nc.tensor.transpose(
                        psin[:, j, :], staged[:, gr + j, :], ident[0:W, 0:W])
                dst_r = ra + gr - r0
                nc.scalar.copy(
                    out=xpad[:, dst_r:dst_r + gn, PAD:PAD + W],
                    in_=psin[:, 0:gn, :])
        return xpad

    def conv_stage(g, xpad):
        """25 taps: DVE handles [0, ldve), PE diag-matmuls the tail."""
        ldve = LS - pe_len
        xflat = xpad[:].rearrange("c h w -> c (h w)")
        acc = acc_pool.tile([128, SH, WP], F32, name="acc", tag="acc")
        aflat = acc[:].rearrange("c h w -> c (h w)")
        for t in range(N * N):
            d = (t // N) * WP + (t % N)
            if t == 0:
                nc.vector.tensor_scalar_mul(
                    aflat[:, 0:ldve], xflat[:, d:d + ldve], kvg[:, g, 0:1])
            else:
                nc.vector.scalar_tensor_tensor(
                    out=aflat[:, 0:ldve],
                    in0=xflat[:, d:d + ldve],
                    scalar=kvg[:, g, t:t + 1],
                    in1=aflat[:, 0:ldve],
                    op0=mybir.AluOpType.mult,
                    op1=mybir.AluOpType.add,
                )
        if not pe_len:
            return acc
        # chunk-groups of 3 with taps outer: one weight load feeds 3
        # matmuls (LDW amortization); each chunk owns a PSUM bank.
        # fp32r inputs must be produced pre-rounded, so each group gets
        # its own small rounded copy (pipelines with the previous group).
        diag = get_diag(g)
        halo = (N - 1) * (WP + 1)
        chunks = [(c0, min(512, LS - c0)) for c0 in range(ldve, LS, 512)]
        for g0 in range(0, len(chunks), 3):
            grp = chunks[g0:g0 + 3]
            glo = grp[0][0]
            gspan = grp[-1][0] + grp[-1][1] - glo + halo
            if pe_f32r:
                xr = xr_pool.tile([128, 3 * 512 + halo], mmdt, name="xr",
                                  tag="xr")
                nc.scalar.copy(out=xr[:, 0:gspan],
                               in_=xflat[:, glo:glo + gspan])
                xpe, off = xr, -glo
            else:
                xpe, off = xflat, 0
            paccs = [pacc_pool.tile([128, 512], F32, name="pacc", tag="pacc")
                     for _ in grp]
            for t in range(N * N):
                d = (t // N) * WP + (t % N)
                for (c0, n), pacc in zip(grp, paccs):
                    nc.tensor.matmul(
                        pacc[:, 0:n],
                        lhsT=diag[:, t, :],
                        rhs=xpe[:, d + c0 + off:d + c0 + off + n],
                        start=(t == 0), stop=(t == N * N - 1),
                    )
            for (c0, n), pacc in zip(grp, paccs):
                nc.scalar.copy(out=aflat[:, c0:c0 + n], in_=pacc[:, 0:n])
        return acc

    def store_stage(g, s, xpad, acc):
        """Gate + residual (in place on acc), transpose back, DMA out."""
        nc.vector.scalar_tensor_tensor(
            out=acc[:, :, 0:W],
            in0=acc[:, :, 0:W],
            scalar=cb[:, 0:1],
            in1=xpad[:, PAD:PAD + SH, PAD:PAD + W],
            op0=mybir.AluOpType.add,
            op1=mybir.AluOpType.mult,
        )
        # transpose back row-wise (4 rows per PSUM bank), stage through
        # SBUF (DMA cannot read PSUM), DMA out 4 output rows at a time
        for r in range(0, SH, 4):
            psout = psout_pool.tile([W, 4, 128], F32, name="psout",
                                    tag="psout")
            for q in range(4):
                nc.tensor.transpose(
                    psout[:, q, :], acc[:, r + q, 0:W], ident[:])
            ostage = ostage_pool.tile([W, 4, 128], F32, name="ostage",
                                      tag="ostage")
            nc.scalar.copy(out=ostage[:], in_=psout[:])
            hr = SH * s + r
            for img, ch_lo, ch_hi, col in SEGS[g]:
                nc.sync.dma_start(
                    out=y_d[img, hr:hr + 4, :, ch_lo:ch_hi].rearrange(
                        "h w c -> w h c"),
                    in_=ostage[:, :, col:col + ch_hi - ch_lo])

    # software pipeline: load(i+1) | conv(i) | gate+store(i-1) per step, so
    # no engine's in-order queue head waits on a cross-stage dependency
    steps = [(g, s) for g in range(NGRP) for s in range(NSTRIP)]
    live = {}
    live[0] = [steps[0], load_stage(*steps[0]), None]
    for i in range(len(steps) + 1):
        if i + 1 < len(steps):
            live[i + 1] = [steps[i + 1], load_stage(*steps[i + 1]), None]
            if pe_len and steps[i + 1][0] != steps[min(i, len(steps) - 1)][0]:
                get_diag(steps[i + 1][0])  # prebuild next group's weights
        # store(i-1) BEFORE conv(i): its transposes-out precede strip i's
        # tap-matmuls in the PE queue, freeing acc slot i-1 early so DVE
        # taps(i+1) don't stall on the slot WAR at the next step
        if i - 1 >= 0:
            (g, s), xpad, acc = live.pop(i - 1)
            store_stage(g, s, xpad, acc)
        if i < len(steps):
            live[i][2] = conv_stage(steps[i][0], live[i][1])
    ctx.close()


_prog_cache = {}


def _get_program(pe_len=None, pe_f32r=None):
    key = (pe_len, pe_f32r)
    if key not in _prog_cache:
        _prog_cache[key] = _build_program(pe_len, pe_f32r)
    return _prog_cache[key]


def _prep_inputs(x, kernel, alpha, bias):
    x = np.ascontiguousarray(np.asarray(x, dtype=np.float32))
    kernel = np.asarray(kernel, dtype=np.float32)
    a = float(np.asarray(alpha).reshape(-1)[0])
    b = float(np.asarray(bias).reshape(-1)[0])
    # kv[g, p, t] = alpha * kernel[tr, tc, (g*128+p) % CH];  cb = bias + 1
    kt = (a * kernel).reshape(N * N, CH).T        # [CH, 25]
    kv = np.ascontiguousarray(
        np.concatenate([kt] * IMG, axis=0).reshape(NGRP, 128, N * N))
    cb = np.full((128, 1), b + 1.0, dtype=np.float32)
    return x, kv.astype(np.float32), cb


def kernel(x, kernel, alpha, bias):
    x, kv, cb = _prep_inputs(x, kernel, alpha, bias)
    nc = _get_program()
    in_maps = [
        {"x": x[c * IMG:(c + 1) * IMG], "kv": kv, "cb": cb}
        for c in range(NCORES)
    ]
    res = run_bass_kernel_spmd(nc, in_maps, list(range(NCORES)))
    out = np.concatenate([res.results[c]["y"] for c in range(NCORES)], axis=0)
    return out
